# revision 31
# baseline (speedup 1.0000x reference)
"""Trainium2 Bass kernel for the gnn_message_passing encoder problem.

kernel(**inputs) takes the FULL inputs and returns the FULL [B, P, R+1] output.

Sharding: 8 cores = 2 batches x 4 object-groups; each core scores 64
(trigger, object) pair slots of one document (group 0 pads its 56 valid
pairs to 64).  The host does layout only (gather of the needed attention
rows, transposes, dtype casts); all arithmetic runs on device.

Device-side layout: sequence positions l ride the SBUF partition dim
(16 tiles of 128).  Per tile the attention rows arrive as [l, (w4 e16 h12)]
so that the span-width sum is two strided adds, the per-pair product
A[s,o,h,l] = Xs[s,h,l]*Xo[o,h,l] is ONE broadcast tensor_tensor mul
(heads innermost keeps every operand packed => DVE 2x mode), and the
head-sum is a small add tree.  The resulting A[l, pair] is directly the
lhsT of the context matmul against seq[l, d] -- no transposes at all on
the main path.  q rides the matmul as an appended ones-column of seq.
Scoring contracts pair embeddings against pre-transposed codebook chunks.
"""

import os
import sys

import numpy as np

for _p in ("/opt/trn_rl_repo", os.path.expanduser("~/.axon_site/_ro/trn_rl_repo")):
    if os.path.isdir(_p) and _p not in sys.path:
        sys.path.insert(0, _p)

import concourse.bass as bass
import concourse.mybir as mybir
import concourse.tile as tile
from concourse import bacc
from concourse.bass_utils import run_bass_kernel_spmd

# Problem dimensions (hardcoded per the harness contract).
B, L, D, H = 2, 2048, 768, 12
E, T, W = 32, 8, 4
R, NN = 57, 20
RN = R + NN            # 77 stacked codebook rows
NE = 16                # entities per core: 8 triggers + 8 objects
NP = 64                # pair slots per core (8 s x 8 o)
NT = 16                # L tiles of 128
NQ = 4                 # quads of 4 tiles
NCORES = 8

# Reference pair order: s-major, o-minor, skip s==o.
ALL_PAIRS = [(s, o) for s in range(T) for o in range(E) if s != o]
GROUP_IDX = [[i for i, (_, o) in enumerate(ALL_PAIRS) if o // 8 == g] for g in range(4)]

F32 = mybir.dt.float32
BF16 = mybir.dt.bfloat16
import ml_dtypes
NP_BF16 = ml_dtypes.bfloat16

LAST_RESULTS = None  # BassKernelResults of the most recent kernel() call


def _build_program():
    nc = bacc.Bacc("TRN2")

    # DRAM inputs (per-core shards, host-prepared).
    attq = nc.dram_tensor("attq", [128, NT * W * NE * H], BF16, kind="ExternalInput")
    seqq = nc.dram_tensor("seqq", [128, NT * (D + 1)], BF16, kind="ExternalInput")
    spansT = nc.dram_tensor("spansT", [128, 6 * NP], BF16, kind="ExternalInput")
    relq = nc.dram_tensor("relq", [128, 18 * RN], BF16, kind="ExternalInput")
    out = nc.dram_tensor("out", [NP, R + 1], F32, kind="ExternalOutput")

    ND = 8               # pipeline unit: duo of 2 L-tiles

    with tile.TileContext(nc) as tc:
        with tc.tile_pool(name="consts", bufs=1) as consts, \
             tc.tile_pool(name="attp", bufs=8) as attp, \
             tc.tile_pool(name="seqp", bufs=4) as seqp, \
             tc.tile_pool(name="work", bufs=2) as work:

            # DMA priority order: the DMA queues are shared FIFOs, so the big
            # seq quads must queue BEHIND all (small, early-needed) att duos.
            # att duos + seq quads all issue from sync; spansT from scalar.
            attv = attq.rearrange("p (t w e h) -> p t w e h", t=NT, w=W, e=NE)
            seqv = seqq.rearrange("p (t d) -> p t d", t=NT)
            att_sbs, seq_sbs = [], []
            for q in range(NQ):
                att_sbs.append(attp.tile([128, 4, W, NE, H], BF16, tag="att", name=f"att{q}"))
            for d in range(ND):
                eng = nc.sync if d % 2 == 0 else nc.scalar
                eng.dma_start(out=att_sbs[d // 2][:, 2 * (d % 2):2 * (d % 2) + 2],
                              in_=attv[:, 2 * d:2 * d + 2])
            spansT_sb = consts.tile([128, 6, NP], BF16)
            nc.scalar.dma_start(out=spansT_sb, in_=spansT.rearrange("p (k n) -> p k n", k=6))
            relq_sb = consts.tile([128, 18, RN], BF16)
            nc.scalar.dma_start(out=relq_sb, in_=relq.rearrange("p (k n) -> p k n", k=18))
            for q in range(NQ):
                seq_sb = seqp.tile([128, 4, D + 1], BF16, tag="seq")
                nc.sync.dma_start(out=seq_sb, in_=seqv[:, 4 * q:4 * q + 4])
                seq_sbs.append(seq_sb)

            A_sb = consts.tile([128, NT, NP, 2], BF16)  # pair rows, heads folded to 2
            id_f32 = consts.tile([RN, RN], F32)
            nc.gpsimd.memset(id_f32, 0.0)
            nc.gpsimd.affine_select(
                out=id_f32, in_=id_f32,
                compare_op=mybir.AluOpType.not_equal, fill=1.0, base=0,
                pattern=[[-1, RN]], channel_multiplier=1,
            )
            # foldI[m, n] = 1 iff m//2 == n: matmul against it transposes a
            # [m, d] chunk AND folds the interleaved (pair, h2) rows.
            foldI = consts.tile([128, NP], BF16)
            nc.gpsimd.memset(foldI, 0.0)
            nc.gpsimd.affine_select(
                out=foldI, in_=foldI,
                compare_op=mybir.AluOpType.not_equal, fill=1.0, base=0,
                pattern=[[-2, NP]], channel_multiplier=1,
            )
            nc.gpsimd.affine_select(
                out=foldI, in_=foldI,
                compare_op=mybir.AluOpType.not_equal, fill=1.0, base=-1,
                pattern=[[-2, NP]], channel_multiplier=1,
            )
            ones_r = consts.tile([1, RN], F32)
            nc.gpsimd.memset(ones_r, 1.0)

            with tc.tile_pool(name="psC", bufs=1, space="PSUM") as psC, \
                 tc.tile_pool(name="psS", bufs=1, space="PSUM") as psS, \
                 tc.tile_pool(name="psT", bufs=1, space="PSUM") as psT:
                # two half-accumulators: tiles 0..7 -> A, 8..15 -> B; the A
                # half is folded + scored mid-chain, only B is on the tail.
                c_psA0 = psC.tile([128, 384], F32, tag="cA0")
                c_psA1 = psC.tile([128, 385], F32, tag="cA1")
                c_psB0 = psC.tile([128, 384], F32, tag="cB0")
                c_psB1 = psC.tile([128, 385], F32, tag="cB1")

                # --- entity scoring side-path (tiny, PE + a couple DVE folds)
                sc_e = psS.tile([RN, 2 * NP], F32, tag="sce")
                for k in range(6):
                    nc.tensor.matmul(out=sc_e[:, 0:NP], lhsT=relq_sb[:, k, :],
                                     rhs=spansT_sb[:, k, :], start=(k == 0), stop=(k == 5))
                for k in range(6):
                    nc.tensor.matmul(out=sc_e[:, NP:2 * NP], lhsT=relq_sb[:, 6 + k, :],
                                     rhs=spansT_sb[:, k, :], start=(k == 0), stop=(k == 5))
                sc_e_sb = consts.tile([RN, 2 * NP], F32)
                nc.scalar.copy(sc_e_sb, sc_e)
                eSO2 = consts.tile([RN, 2, NE, 2], F32)
                eSO = consts.tile([RN, 2, NE], F32)
                v = sc_e_sb.rearrange("r (x e w) -> r x e w", x=2, w=W)
                nc.vector.tensor_add(eSO2, v[:, :, :, 0:2], v[:, :, :, 2:4])
                nc.vector.tensor_add(eSO, eSO2[:, :, :, 0], eSO2[:, :, :, 1])
                eSum = consts.tile([RN, T, 8], F32)     # 0.25*(eS[s]+eO[o])
                nc.vector.scalar_tensor_tensor(
                    out=eSum,
                    in0=eSO[:, 0, 0:T].unsqueeze(2).broadcast_to([RN, T, 8]),
                    scalar=1.0,
                    in1=eSO[:, 1, T:NE].unsqueeze(1).broadcast_to([RN, T, 8]),
                    op0=mybir.AluOpType.mult, op1=mybir.AluOpType.add,
                )
                nc.vector.tensor_scalar_mul(eSum, eSum, 0.25)

                c2 = consts.tile([128, 2, D + 1], BF16)   # [.., 0]=A, [.., 1]=B
                cT_fA = consts.tile([128, 6, NP], BF16)
                cT_fB = consts.tile([128, 6, NP], BF16)
                sc_c = psS.tile([RN, NP], F32, tag="sce")

                # --- main path: per quad of 4 L-tiles
                for qd in range(NQ):
                    att_sb = att_sbs[qd]
                    # span-width sum: (w01)+(w23)  [packed, 2x]
                    wf = work.tile([128, 4, 2, NE * H], BF16, tag="wf")
                    nc.vector.tensor_add(
                        wf,
                        att_sb.rearrange("p t w e h -> p t w (e h)")[:, :, 0:2],
                        att_sb.rearrange("p t w e h -> p t w (e h)")[:, :, 2:4],
                    )
                    X = work.tile([128, 4, NE, H], BF16, tag="X")
                    nc.vector.tensor_add(
                        X.rearrange("p t e h -> p t (e h)"),
                        wf[:, :, 0], wf[:, :, 1],
                    )
                    # pair products (broadcast APs; 3 free dims max per op)
                    prod = work.tile([128, 4, T, 8, H], BF16, tag="prod")
                    for i in range(4):
                        nc.vector.tensor_mul(
                            prod[:, i],
                            X[:, i, 0:T, :].unsqueeze(2).broadcast_to([128, T, 8, H]),
                            X[:, i, T:NE, :].unsqueeze(1).broadcast_to([128, T, 8, H]),
                        )
                    # head-sum 12 -> 6 -> 2 (last 2-fold rides the c matmul M)
                    h6 = work.tile([128, 4, NP, 6], BF16, tag="h6")
                    pv = prod.rearrange("p t s o h -> p t (s o) h")
                    nc.vector.tensor_add(h6, pv[:, :, :, 0:6], pv[:, :, :, 6:12])
                    Ad = A_sb[:, 4 * qd:4 * qd + 4]
                    nc.vector.tensor_add(Ad, h6[:, :, :, 0:2], h6[:, :, :, 2:4])
                    nc.vector.tensor_add(Ad, Ad, h6[:, :, :, 4:6])

                    # context matmul accumulation (M = 128 = (pair, h2))
                    for i in range(4):
                        t = 4 * qd + i
                        ps0, ps1 = (c_psA0, c_psA1) if t < 8 else (c_psB0, c_psB1)
                        lhs = A_sb[:, t].rearrange("p n h -> p (n h)")
                        sq = seq_sbs[t // 4][:, t % 4]
                        nc.tensor.matmul(out=ps0, lhsT=lhs, rhs=sq[:, 0:384],
                                         start=(t % 8 == 0), stop=(t % 8 == 7))
                        nc.tensor.matmul(out=ps1, lhsT=lhs, rhs=sq[:, 384:769],
                                         start=(t % 8 == 0), stop=(t % 8 == 7))

                    if qd == 1:  # A half: evacuate, fold, and score mid-chain
                        nc.scalar.copy(c2[:, 0, 0:384], c_psA0)
                        nc.scalar.copy(c2[:, 0, 384:769], c_psA1)
                        cT_psA = psT.tile([128, 6, NP], F32, tag="cT")
                        for k in range(6):
                            nc.tensor.matmul(out=cT_psA[:, k, :],
                                             lhsT=c2[:, 0, 128 * k:128 * (k + 1)],
                                             rhs=foldI)
                        nc.scalar.copy(cT_fA, cT_psA)
                        for k in range(6):
                            nc.tensor.matmul(out=sc_c, lhsT=relq_sb[:, 12 + k, :],
                                             rhs=cT_fA[:, k, :],
                                             start=(k == 0), stop=False)

                # --- tail: B half, 1/q, final assembly
                nc.scalar.copy(c2[:, 1, 0:384], c_psB0)
                nc.vector.tensor_copy(c2[:, 1, 384:769], c_psB1)
                # q chain (parallel to the big fold): q = qA + qB, transposed
                # and pair-folded by one tiny matmul against foldI
                qsum = consts.tile([128, 1], BF16)
                nc.vector.tensor_add(qsum, c2[:, 0, 768:769], c2[:, 1, 768:769])
                qT_ps = psT.tile([1, NP], F32, tag="qT")
                nc.tensor.matmul(out=qT_ps, lhsT=qsum, rhs=foldI)
                rq_row = consts.tile([1, NP], F32)
                nc.vector.reciprocal_approx_fast(rq_row, qT_ps)
                # B-half fold + score accumulation (before the rq replication
                # so the PE stream reaches sc_c as early as possible)
                cT_psB = psT.tile([128, 6, NP], F32, tag="cT")
                for k in range(6):
                    nc.tensor.matmul(out=cT_psB[:, k, :],
                                     lhsT=c2[:, 1, 128 * k:128 * (k + 1)],
                                     rhs=foldI)
                nc.vector.tensor_copy(cT_fB, cT_psB)
                for k in range(6):
                    nc.tensor.matmul(out=sc_c, lhsT=relq_sb[:, 12 + k, :],
                                     rhs=cT_fB[:, k, :],
                                     start=False, stop=(k == 5))
                rq_ps = psT.tile([RN, NP], F32, tag="qT")
                nc.tensor.matmul(out=rq_ps, lhsT=ones_r, rhs=rq_row)
                rq_rep = consts.tile([RN, NP], BF16)
                nc.scalar.copy(rq_rep, rq_ps)
                fin_T = consts.tile([RN, NP], F32)
                # (c_scores * rq) + precomputed 0.25*(eS[s(p)] + eO[o(p)])
                nc.vector.tensor_mul(fin_T, sc_c, rq_rep)
                fin_T2 = consts.tile([RN, NP], F32)
                nc.vector.tensor_add(fin_T2.rearrange("r (s o) -> r s o", s=T),
                                     fin_T.rearrange("r (s o) -> r s o", s=T),
                                     eSum)

                # transpose to [p, rn], NOTA max, assemble [p, 1+R]
                finT_ps = psT.tile([NP, RN], F32, tag="fT")
                nc.tensor.transpose(finT_ps, fin_T2, id_f32)
                res = consts.tile([NP, R + 1], F32)
                nc.vector.reduce_max(res[:, 0:1], finT_ps[:, R:RN],
                                     axis=mybir.AxisListType.X)
                nc.scalar.copy(res[:, 1:R + 1], finT_ps[:, 0:R])
                nc.sync.dma_start(out=out[:, :], in_=res)

    return nc


def _host_shards(sequence_output, attention, relation_embeddings, nota_embeddings,
                 span_starts):
    rel_all = np.concatenate(
        [np.asarray(relation_embeddings, np.float32),
         np.asarray(nota_embeddings, np.float32)], axis=0)          # [77, 2304]
    relq = np.ascontiguousarray(
        rel_all.T.reshape(18, 128, RN).transpose(1, 0, 2).reshape(128, 18 * RN)
    ).astype(NP_BF16)

    in_maps = []
    for c in range(NCORES):
        b, g = divmod(c, 4)
        obj = list(range(8)) if g == 0 else list(range(8 * g, 8 * g + 8))
        ents = list(range(T)) + obj
        rows = np.concatenate(
            [np.arange(span_starts[b, e], span_starts[b, e] + W) for e in ents]
        )
        # attention rows -> [l, w, e, h] -> [128, (t w e h)]
        att_rows = attention[b][:, rows, :]                          # [H, 64, L]
        att_t = att_rows.reshape(H, NE, W, L).transpose(3, 2, 1, 0)  # [L, w, e, h]
        attq = (att_t.reshape(NT, 128, W * NE * H).transpose(1, 0, 2)
                .reshape(128, NT * W * NE * H)).astype(NP_BF16)
        # seq tiles + ones column -> [128, (t, 769)]
        st = sequence_output[b].reshape(NT, 128, D)
        st = np.concatenate([st, np.ones((NT, 128, 1), np.float32)], axis=2)
        seqq = st.transpose(1, 0, 2).reshape(128, NT * (D + 1)).astype(NP_BF16)
        # span rows transposed -> [d, row] chunks [128, (6, 64)]
        spT = sequence_output[b][rows].T.reshape(6, 128, NP).transpose(1, 0, 2)
        spansT = spT.reshape(128, 6 * NP).astype(NP_BF16)
        in_maps.append({
            "attq": np.ascontiguousarray(attq),
            "seqq": np.ascontiguousarray(seqq),
            "spansT": np.ascontiguousarray(spansT),
            "relq": relq,
        })
    return in_maps


def kernel(sequence_output, attention, relation_embeddings, nota_embeddings,
           span_starts):
    global LAST_RESULTS
    sequence_output = np.asarray(sequence_output, np.float32)
    attention = np.asarray(attention, np.float32)
    span_starts = np.asarray(span_starts)

    in_maps = _host_shards(sequence_output, attention, relation_embeddings,
                           nota_embeddings, span_starts)

    nc = _build_program()
    nc.finalize()
    LAST_RESULTS = run_bass_kernel_spmd(nc, in_maps, core_ids=list(range(NCORES)))

    out = np.zeros((B, len(ALL_PAIRS), R + 1), np.float32)
    for c in range(NCORES):
        b, g = divmod(c, 4)
        res = LAST_RESULTS.results[c]["out"]          # [64, 78], p = s*8+o_local
        idxs = GROUP_IDX[g]
        rows = [s * 8 + (o % 8) for (s, o) in (ALL_PAIRS[i] for i in idxs)]
        out[b, idxs, :] = res[rows]
    return out


# revision 32
# speedup vs baseline: 1.0744x; 1.0744x over previous
"""Trainium2 Bass kernel for the gnn_message_passing encoder problem.

kernel(**inputs) takes the FULL inputs and returns the FULL [B, P, R+1] output.

Sharding: 8 cores = 2 batches x 4 object-groups; each core scores 64
(trigger, object) pair slots of one document (group 0 pads its 56 valid
pairs to 64).  The host does layout only (gather of the needed attention
rows, transposes, dtype casts); all arithmetic runs on device.

Device-side layout: sequence positions l ride the SBUF partition dim
(16 tiles of 128).  Per tile the attention rows arrive as [l, (w4 e16 h12)]
so that the span-width sum is two strided adds, the per-pair product
A[s,o,h,l] = Xs[s,h,l]*Xo[o,h,l] is ONE broadcast tensor_tensor mul
(heads innermost keeps every operand packed => DVE 2x mode), and the
head-sum is a small add tree.  The resulting A[l, pair] is directly the
lhsT of the context matmul against seq[l, d] -- no transposes at all on
the main path.  q rides the matmul as an appended ones-column of seq.
Scoring contracts pair embeddings against pre-transposed codebook chunks.
"""

import os
import sys

import numpy as np

for _p in ("/opt/trn_rl_repo", os.path.expanduser("~/.axon_site/_ro/trn_rl_repo")):
    if os.path.isdir(_p) and _p not in sys.path:
        sys.path.insert(0, _p)

import concourse.bass as bass
import concourse.mybir as mybir
import concourse.tile as tile
from concourse import bacc
from concourse.bass_utils import run_bass_kernel_spmd

# Problem dimensions (hardcoded per the harness contract).
B, L, D, H = 2, 2048, 768, 12
E, T, W = 32, 8, 4
R, NN = 57, 20
RN = R + NN            # 77 stacked codebook rows
NE = 16                # entities per core: 8 triggers + 8 objects
NP = 64                # pair slots per core (8 s x 8 o)
NT = 16                # L tiles of 128
NQ = 4                 # quads of 4 tiles
NCORES = 8

# Reference pair order: s-major, o-minor, skip s==o.
ALL_PAIRS = [(s, o) for s in range(T) for o in range(E) if s != o]
GROUP_IDX = [[i for i, (_, o) in enumerate(ALL_PAIRS) if o // 8 == g] for g in range(4)]

F32 = mybir.dt.float32
BF16 = mybir.dt.bfloat16
import ml_dtypes
NP_BF16 = ml_dtypes.bfloat16

LAST_RESULTS = None  # BassKernelResults of the most recent kernel() call


def _build_program():
    nc = bacc.Bacc("TRN2")

    # DRAM inputs (per-core shards, host-prepared).
    attq = nc.dram_tensor("attq", [128, NT * W * NE * H], BF16, kind="ExternalInput")
    seqq = nc.dram_tensor("seqq", [128, NT * (D + 1)], BF16, kind="ExternalInput")
    spansT = nc.dram_tensor("spansT", [128, 6 * NP], BF16, kind="ExternalInput")
    relq = nc.dram_tensor("relq", [128, 18 * RN], BF16, kind="ExternalInput")
    out = nc.dram_tensor("out", [NP, R + 1], F32, kind="ExternalOutput")

    ND = 8               # pipeline unit: duo of 2 L-tiles

    with tile.TileContext(nc) as tc:
        with tc.tile_pool(name="consts", bufs=1) as consts, \
             tc.tile_pool(name="attp", bufs=8) as attp, \
             tc.tile_pool(name="seqp", bufs=4) as seqp, \
             tc.tile_pool(name="work", bufs=2) as work:

            # DMA priority order: the DMA queues are shared FIFOs, so the big
            # seq quads must queue BEHIND all (small, early-needed) att duos.
            # att duos + seq quads all issue from sync; spansT from scalar.
            attv = attq.rearrange("p (t w e h) -> p t w e h", t=NT, w=W, e=NE)
            seqv = seqq.rearrange("p (t d) -> p t d", t=NT)
            att_sbs, seq_sbs = [], []
            for q in range(NQ):
                att_sbs.append(attp.tile([128, 4, W, NE, H], BF16, tag="att", name=f"att{q}"))
            for d in range(2):
                eng = nc.sync if d % 2 == 0 else nc.scalar
                eng.dma_start(out=att_sbs[d // 2][:, 2 * (d % 2):2 * (d % 2) + 2],
                              in_=attv[:, 2 * d:2 * d + 2])
            spansT_sb = consts.tile([128, 6, NP], BF16)
            nc.scalar.dma_start(out=spansT_sb, in_=spansT.rearrange("p (k n) -> p k n", k=6))
            relq_sb = consts.tile([128, 18, RN], BF16)
            nc.scalar.dma_start(out=relq_sb, in_=relq.rearrange("p (k n) -> p k n", k=18))
            for d in range(2, ND):
                eng = nc.sync if d % 2 == 0 else nc.scalar
                eng.dma_start(out=att_sbs[d // 2][:, 2 * (d % 2):2 * (d % 2) + 2],
                              in_=attv[:, 2 * d:2 * d + 2])
            for q in range(NQ):
                seq_sb = seqp.tile([128, 4, D + 1], BF16, tag="seq")
                nc.sync.dma_start(out=seq_sb, in_=seqv[:, 4 * q:4 * q + 4])
                seq_sbs.append(seq_sb)

            A_sb = consts.tile([128, NT, NP, 2], BF16)  # pair rows, heads folded to 2
            id_f32 = consts.tile([RN, RN], F32)
            nc.gpsimd.memset(id_f32, 0.0)
            nc.gpsimd.affine_select(
                out=id_f32, in_=id_f32,
                compare_op=mybir.AluOpType.not_equal, fill=1.0, base=0,
                pattern=[[-1, RN]], channel_multiplier=1,
            )
            # foldI[m, n] = 1 iff m//2 == n: matmul against it transposes a
            # [m, d] chunk AND folds the interleaved (pair, h2) rows.
            foldI = consts.tile([128, NP], BF16)
            nc.gpsimd.memset(foldI, 0.0)
            nc.gpsimd.affine_select(
                out=foldI, in_=foldI,
                compare_op=mybir.AluOpType.not_equal, fill=1.0, base=0,
                pattern=[[-2, NP]], channel_multiplier=1,
            )
            nc.gpsimd.affine_select(
                out=foldI, in_=foldI,
                compare_op=mybir.AluOpType.not_equal, fill=1.0, base=-1,
                pattern=[[-2, NP]], channel_multiplier=1,
            )
            ones_r = consts.tile([1, RN], F32)
            nc.gpsimd.memset(ones_r, 1.0)

            with tc.tile_pool(name="psC", bufs=1, space="PSUM") as psC, \
                 tc.tile_pool(name="psS", bufs=1, space="PSUM") as psS, \
                 tc.tile_pool(name="psT", bufs=1, space="PSUM") as psT:
                # two half-accumulators: tiles 0..7 -> A, 8..15 -> B; the A
                # half is folded + scored mid-chain, only B is on the tail.
                c_psA0 = psC.tile([128, 384], F32, tag="cA0")
                c_psA1 = psC.tile([128, 385], F32, tag="cA1")
                c_psB0 = psC.tile([128, 384], F32, tag="cB0")
                c_psB1 = psC.tile([128, 385], F32, tag="cB1")

                # --- entity scoring side-path (tiny, PE + a couple DVE folds)
                sc_e = psS.tile([RN, 2 * NP], F32, tag="sce")
                for k in range(6):
                    nc.tensor.matmul(out=sc_e[:, 0:NP], lhsT=relq_sb[:, k, :],
                                     rhs=spansT_sb[:, k, :], start=(k == 0), stop=(k == 5))
                for k in range(6):
                    nc.tensor.matmul(out=sc_e[:, NP:2 * NP], lhsT=relq_sb[:, 6 + k, :],
                                     rhs=spansT_sb[:, k, :], start=(k == 0), stop=(k == 5))

                c2 = consts.tile([128, 2, D + 1], BF16)   # [.., 0]=A, [.., 1]=B
                cT_fA = consts.tile([128, 6, NP], BF16)
                cT_fB = consts.tile([128, 6, NP], BF16)
                sc_c = psS.tile([RN, NP], F32, tag="sce")

                # --- main path: per quad of 4 L-tiles
                for qd in range(NQ):
                    att_sb = att_sbs[qd]
                    # span-width sum: (w01)+(w23)  [packed, 2x]
                    wf = work.tile([128, 4, 2, NE * H], BF16, tag="wf")
                    nc.vector.tensor_add(
                        wf,
                        att_sb.rearrange("p t w e h -> p t w (e h)")[:, :, 0:2],
                        att_sb.rearrange("p t w e h -> p t w (e h)")[:, :, 2:4],
                    )
                    X = work.tile([128, 4, NE, H], BF16, tag="X")
                    nc.vector.tensor_add(
                        X.rearrange("p t e h -> p t (e h)"),
                        wf[:, :, 0], wf[:, :, 1],
                    )
                    # pair products (broadcast APs; 3 free dims max per op)
                    prod = work.tile([128, 4, T, 8, H], BF16, tag="prod")
                    for i in range(4):
                        nc.vector.tensor_mul(
                            prod[:, i],
                            X[:, i, 0:T, :].unsqueeze(2).broadcast_to([128, T, 8, H]),
                            X[:, i, T:NE, :].unsqueeze(1).broadcast_to([128, T, 8, H]),
                        )
                    # head-sum 12 -> 6 -> 2 (last 2-fold rides the c matmul M)
                    h6 = work.tile([128, 4, NP, 6], BF16, tag="h6")
                    pv = prod.rearrange("p t s o h -> p t (s o) h")
                    nc.vector.tensor_add(h6, pv[:, :, :, 0:6], pv[:, :, :, 6:12])
                    Ad = A_sb[:, 4 * qd:4 * qd + 4]
                    nc.vector.tensor_add(Ad, h6[:, :, :, 0:2], h6[:, :, :, 2:4])
                    nc.vector.tensor_add(Ad, Ad, h6[:, :, :, 4:6])

                    # context matmul accumulation (M = 128 = (pair, h2))
                    for i in range(4):
                        t = 4 * qd + i
                        ps0, ps1 = (c_psA0, c_psA1) if t < 8 else (c_psB0, c_psB1)
                        lhs = A_sb[:, t].rearrange("p n h -> p (n h)")
                        sq = seq_sbs[t // 4][:, t % 4]
                        nc.tensor.matmul(out=ps0, lhsT=lhs, rhs=sq[:, 0:384],
                                         start=(t % 8 == 0), stop=(t % 8 == 7))
                        nc.tensor.matmul(out=ps1, lhsT=lhs, rhs=sq[:, 384:769],
                                         start=(t % 8 == 0), stop=(t % 8 == 7))

                    if qd == 1:  # A half: evacuate, fold, and score mid-chain
                        nc.scalar.copy(c2[:, 0, 0:384], c_psA0)
                        nc.scalar.copy(c2[:, 0, 384:769], c_psA1)
                        cT_psA = psT.tile([128, 6, NP], F32, tag="cT")
                        for k in range(6):
                            nc.tensor.matmul(out=cT_psA[:, k, :],
                                             lhsT=c2[:, 0, 128 * k:128 * (k + 1)],
                                             rhs=foldI)
                        nc.scalar.copy(cT_fA, cT_psA)
                        for k in range(6):
                            nc.tensor.matmul(out=sc_c, lhsT=relq_sb[:, 12 + k, :],
                                             rhs=cT_fA[:, k, :],
                                             start=(k == 0), stop=False)

                    if qd == 1:  # entity-score folds (emitted late so the
                        # in-order DVE stream cannot block early on them)
                        sc_e_sb = consts.tile([RN, 2 * NP], F32)
                        nc.scalar.copy(sc_e_sb, sc_e)
                        eSO2 = consts.tile([RN, 2, NE, 2], F32)
                        eSO = consts.tile([RN, 2, NE], F32)
                        v = sc_e_sb.rearrange("r (x e w) -> r x e w", x=2, w=W)
                        nc.vector.tensor_add(eSO2, v[:, :, :, 0:2], v[:, :, :, 2:4])
                        nc.vector.tensor_add(eSO, eSO2[:, :, :, 0], eSO2[:, :, :, 1])
                        eSum = consts.tile([RN, T, 8], F32)     # 0.25*(eS[s]+eO[o])
                        nc.vector.scalar_tensor_tensor(
                            out=eSum,
                            in0=eSO[:, 0, 0:T].unsqueeze(2).broadcast_to([RN, T, 8]),
                            scalar=1.0,
                            in1=eSO[:, 1, T:NE].unsqueeze(1).broadcast_to([RN, T, 8]),
                            op0=mybir.AluOpType.mult, op1=mybir.AluOpType.add,
                        )
                        nc.vector.tensor_scalar_mul(eSum, eSum, 0.25)


                # --- tail: B half, 1/q, final assembly
                nc.scalar.copy(c2[:, 1, 0:384], c_psB0)
                nc.vector.tensor_copy(c2[:, 1, 384:769], c_psB1)
                # q chain (parallel to the big fold): q = qA + qB, transposed
                # and pair-folded by one tiny matmul against foldI
                qsum = consts.tile([128, 1], BF16)
                nc.vector.tensor_add(qsum, c2[:, 0, 768:769], c2[:, 1, 768:769])
                qT_ps = psT.tile([1, NP], F32, tag="qT")
                nc.tensor.matmul(out=qT_ps, lhsT=qsum, rhs=foldI)
                rq_row = consts.tile([1, NP], F32)
                nc.vector.reciprocal_approx_fast(rq_row, qT_ps)
                # B-half fold + score accumulation (before the rq replication
                # so the PE stream reaches sc_c as early as possible)
                cT_psB = psT.tile([128, 6, NP], F32, tag="cT")
                for k in range(6):
                    nc.tensor.matmul(out=cT_psB[:, k, :],
                                     lhsT=c2[:, 1, 128 * k:128 * (k + 1)],
                                     rhs=foldI)
                nc.vector.tensor_copy(cT_fB, cT_psB)
                for k in range(6):
                    nc.tensor.matmul(out=sc_c, lhsT=relq_sb[:, 12 + k, :],
                                     rhs=cT_fB[:, k, :],
                                     start=False, stop=(k == 5))
                rq_ps = psT.tile([RN, NP], F32, tag="qT")
                nc.tensor.matmul(out=rq_ps, lhsT=ones_r, rhs=rq_row)
                rq_rep = consts.tile([RN, NP], BF16)
                nc.scalar.copy(rq_rep, rq_ps)
                fin_T = consts.tile([RN, NP], F32)
                # (c_scores * rq) + precomputed 0.25*(eS[s(p)] + eO[o(p)])
                nc.vector.tensor_mul(fin_T, sc_c, rq_rep)
                fin_T2 = consts.tile([RN, NP], F32)
                nc.vector.tensor_add(fin_T2.rearrange("r (s o) -> r s o", s=T),
                                     fin_T.rearrange("r (s o) -> r s o", s=T),
                                     eSum)

                # transpose to [p, rn], NOTA max, assemble [p, 1+R]
                finT_ps = psT.tile([NP, RN], F32, tag="fT")
                nc.tensor.transpose(finT_ps, fin_T2, id_f32)
                res = consts.tile([NP, R + 1], F32)
                nc.vector.reduce_max(res[:, 0:1], finT_ps[:, R:RN],
                                     axis=mybir.AxisListType.X)
                nc.scalar.copy(res[:, 1:R + 1], finT_ps[:, 0:R])
                nc.sync.dma_start(out=out[:, :], in_=res)

    return nc


def _host_shards(sequence_output, attention, relation_embeddings, nota_embeddings,
                 span_starts):
    rel_all = np.concatenate(
        [np.asarray(relation_embeddings, np.float32),
         np.asarray(nota_embeddings, np.float32)], axis=0)          # [77, 2304]
    relq = np.ascontiguousarray(
        rel_all.T.reshape(18, 128, RN).transpose(1, 0, 2).reshape(128, 18 * RN)
    ).astype(NP_BF16)

    in_maps = []
    for c in range(NCORES):
        b, g = divmod(c, 4)
        obj = list(range(8)) if g == 0 else list(range(8 * g, 8 * g + 8))
        ents = list(range(T)) + obj
        rows = np.concatenate(
            [np.arange(span_starts[b, e], span_starts[b, e] + W) for e in ents]
        )
        # attention rows -> [l, w, e, h] -> [128, (t w e h)]
        att_rows = attention[b][:, rows, :]                          # [H, 64, L]
        att_t = att_rows.reshape(H, NE, W, L).transpose(3, 2, 1, 0)  # [L, w, e, h]
        attq = (att_t.reshape(NT, 128, W * NE * H).transpose(1, 0, 2)
                .reshape(128, NT * W * NE * H)).astype(NP_BF16)
        # seq tiles + ones column -> [128, (t, 769)]
        st = sequence_output[b].reshape(NT, 128, D)
        st = np.concatenate([st, np.ones((NT, 128, 1), np.float32)], axis=2)
        seqq = st.transpose(1, 0, 2).reshape(128, NT * (D + 1)).astype(NP_BF16)
        # span rows transposed -> [d, row] chunks [128, (6, 64)]
        spT = sequence_output[b][rows].T.reshape(6, 128, NP).transpose(1, 0, 2)
        spansT = spT.reshape(128, 6 * NP).astype(NP_BF16)
        in_maps.append({
            "attq": np.ascontiguousarray(attq),
            "seqq": np.ascontiguousarray(seqq),
            "spansT": np.ascontiguousarray(spansT),
            "relq": relq,
        })
    return in_maps


def kernel(sequence_output, attention, relation_embeddings, nota_embeddings,
           span_starts):
    global LAST_RESULTS
    sequence_output = np.asarray(sequence_output, np.float32)
    attention = np.asarray(attention, np.float32)
    span_starts = np.asarray(span_starts)

    in_maps = _host_shards(sequence_output, attention, relation_embeddings,
                           nota_embeddings, span_starts)

    nc = _build_program()
    nc.finalize()
    LAST_RESULTS = run_bass_kernel_spmd(nc, in_maps, core_ids=list(range(NCORES)))

    out = np.zeros((B, len(ALL_PAIRS), R + 1), np.float32)
    for c in range(NCORES):
        b, g = divmod(c, 4)
        res = LAST_RESULTS.results[c]["out"]          # [64, 78], p = s*8+o_local
        idxs = GROUP_IDX[g]
        rows = [s * 8 + (o % 8) for (s, o) in (ALL_PAIRS[i] for i in idxs)]
        out[b, idxs, :] = res[rows]
    return out


# revision 33
# speedup vs baseline: 1.2083x; 1.1246x over previous
"""Trainium2 Bass kernel for the gnn_message_passing encoder problem.

kernel(**inputs) takes the FULL inputs and returns the FULL [B, P, R+1] output.

Sharding: 8 cores = 2 batches x 4 object-groups; each core scores 64
(trigger, object) pair slots of one document (group 0 pads its 56 valid
pairs to 64).  The host does layout only (gather of the needed attention
rows, transposes, dtype casts); all arithmetic runs on device.

Device-side layout: sequence positions l ride the SBUF partition dim
(16 tiles of 128).  Per tile the attention rows arrive as [l, (w4 e16 h12)]
so that the span-width sum is two strided adds, the per-pair product
A[s,o,h,l] = Xs[s,h,l]*Xo[o,h,l] is ONE broadcast tensor_tensor mul
(heads innermost keeps every operand packed => DVE 2x mode), and the
head-sum is a small add tree.  The resulting A[l, pair] is directly the
lhsT of the context matmul against seq[l, d] -- no transposes at all on
the main path.  q rides the matmul as an appended ones-column of seq.
Scoring contracts pair embeddings against pre-transposed codebook chunks.
"""

import os
import sys

import numpy as np

for _p in ("/opt/trn_rl_repo", os.path.expanduser("~/.axon_site/_ro/trn_rl_repo")):
    if os.path.isdir(_p) and _p not in sys.path:
        sys.path.insert(0, _p)

import concourse.bass as bass
import concourse.mybir as mybir
import concourse.tile as tile
from concourse import bacc
from concourse.bass_utils import run_bass_kernel_spmd

# Problem dimensions (hardcoded per the harness contract).
B, L, D, H = 2, 2048, 768, 12
E, T, W = 32, 8, 4
R, NN = 57, 20
RN = R + NN            # 77 stacked codebook rows
NE = 16                # entities per core: 8 triggers + 8 objects
NP = 64                # pair slots per core (8 s x 8 o)
NT = 16                # L tiles of 128
NQ = 4                 # quads of 4 tiles
NCORES = 8

# Reference pair order: s-major, o-minor, skip s==o.
ALL_PAIRS = [(s, o) for s in range(T) for o in range(E) if s != o]
GROUP_IDX = [[i for i, (_, o) in enumerate(ALL_PAIRS) if o // 8 == g] for g in range(4)]

F32 = mybir.dt.float32
BF16 = mybir.dt.bfloat16
import ml_dtypes
NP_BF16 = ml_dtypes.bfloat16

LAST_RESULTS = None  # BassKernelResults of the most recent kernel() call


def _build_program():
    nc = bacc.Bacc("TRN2")

    # DRAM inputs (per-core shards, host-prepared).
    attq = nc.dram_tensor("attq", [128, NT * W * NE * H], BF16, kind="ExternalInput")
    seqq = nc.dram_tensor("seqq", [128, NT * (D + 1)], BF16, kind="ExternalInput")
    spansT = nc.dram_tensor("spansT", [128, 6 * NP], BF16, kind="ExternalInput")
    relq = nc.dram_tensor("relq", [128, 18 * RN], BF16, kind="ExternalInput")
    out = nc.dram_tensor("out", [NP, R + 1], F32, kind="ExternalOutput")

    ND = 8               # pipeline unit: duo of 2 L-tiles

    with tile.TileContext(nc) as tc:
        with tc.tile_pool(name="consts", bufs=1) as consts, \
             tc.tile_pool(name="attp", bufs=8) as attp, \
             tc.tile_pool(name="seqp", bufs=4) as seqp, \
             tc.tile_pool(name="work", bufs=2) as work:

            # DMA priority order: the DMA queues are shared FIFOs, so the big
            # seq quads must queue BEHIND all (small, early-needed) att duos.
            # att duos + seq quads all issue from sync; spansT/relq (small,
            # needed by the mid-chain entity-score path) from scalar.
            attv = attq.rearrange("p (t w e h) -> p t w e h", t=NT, w=W, e=NE)
            seqv = seqq.rearrange("p (t d) -> p t d", t=NT)
            att_sbs, seq_sbs = [], []
            for d in range(ND):
                att_sb = attp.tile([128, 2, W, NE, H], BF16, tag="att")
                nc.sync.dma_start(out=att_sb, in_=attv[:, 2 * d:2 * d + 2])
                att_sbs.append(att_sb)
            spansT_sb = consts.tile([128, 6, NP], BF16)
            nc.scalar.dma_start(out=spansT_sb, in_=spansT.rearrange("p (k n) -> p k n", k=6))
            relq_sb = consts.tile([128, 18, RN], BF16)
            nc.scalar.dma_start(out=relq_sb, in_=relq.rearrange("p (k n) -> p k n", k=18))
            for q in range(NQ):
                seq_sb = seqp.tile([128, 4, D + 1], BF16, tag="seq")
                nc.sync.dma_start(out=seq_sb, in_=seqv[:, 4 * q:4 * q + 4])
                seq_sbs.append(seq_sb)

            A_sb = consts.tile([128, NT, NP, 2], BF16)  # pair rows, heads folded to 2
            id_f32 = consts.tile([RN, RN], F32)
            nc.gpsimd.memset(id_f32, 0.0)
            nc.gpsimd.affine_select(
                out=id_f32, in_=id_f32,
                compare_op=mybir.AluOpType.not_equal, fill=1.0, base=0,
                pattern=[[-1, RN]], channel_multiplier=1,
            )
            # foldI[m, n] = 1 iff m//2 == n: matmul against it transposes a
            # [m, d] chunk AND folds the interleaved (pair, h2) rows.
            foldI = consts.tile([128, NP], BF16)
            nc.gpsimd.memset(foldI, 0.0)
            nc.gpsimd.affine_select(
                out=foldI, in_=foldI,
                compare_op=mybir.AluOpType.not_equal, fill=1.0, base=0,
                pattern=[[-2, NP]], channel_multiplier=1,
            )
            nc.gpsimd.affine_select(
                out=foldI, in_=foldI,
                compare_op=mybir.AluOpType.not_equal, fill=1.0, base=-1,
                pattern=[[-2, NP]], channel_multiplier=1,
            )
            ones_r = consts.tile([1, RN], F32)
            nc.gpsimd.memset(ones_r, 1.0)

            with tc.tile_pool(name="psC", bufs=1, space="PSUM") as psC, \
                 tc.tile_pool(name="psS", bufs=1, space="PSUM") as psS, \
                 tc.tile_pool(name="psT", bufs=1, space="PSUM") as psT:
                # two half-accumulators: tiles 0..7 -> A, 8..15 -> B; the A
                # half is folded + scored mid-chain, only B is on the tail.
                c_psA0 = psC.tile([128, 384], F32, tag="cA0")
                c_psA1 = psC.tile([128, 385], F32, tag="cA1")
                c_psB0 = psC.tile([128, 384], F32, tag="cB0")
                c_psB1 = psC.tile([128, 385], F32, tag="cB1")

                # --- entity scoring side-path (PE part; DVE folds are emitted
                # mid-loop so the in-order DVE stream cannot block on them)
                sc_e = psS.tile([RN, 2 * NP], F32, tag="sce")
                for k in range(6):
                    nc.tensor.matmul(out=sc_e[:, 0:NP], lhsT=relq_sb[:, k, :],
                                     rhs=spansT_sb[:, k, :], start=(k == 0), stop=(k == 5))
                for k in range(6):
                    nc.tensor.matmul(out=sc_e[:, NP:2 * NP], lhsT=relq_sb[:, 6 + k, :],
                                     rhs=spansT_sb[:, k, :], start=(k == 0), stop=(k == 5))

                c2 = consts.tile([128, 2, D + 1], BF16)   # [.., 0]=A, [.., 1]=B
                cT_fA = consts.tile([128, 6, NP], BF16)
                cT_fB = consts.tile([128, 6, NP], BF16)
                sc_c = psS.tile([RN, NP], F32, tag="sce")

                # --- main path: per duo of 2 L-tiles
                for d in range(ND):
                    att_sb = att_sbs[d]
                    # span-width sum: (w01)+(w23)  [packed, 2x]
                    wf = work.tile([128, 2, 2, NE * H], BF16, tag="wf")
                    nc.vector.tensor_add(
                        wf,
                        att_sb.rearrange("p t w e h -> p t w (e h)")[:, :, 0:2],
                        att_sb.rearrange("p t w e h -> p t w (e h)")[:, :, 2:4],
                    )
                    X = work.tile([128, 2, NE, H], BF16, tag="X")
                    nc.vector.tensor_add(
                        X.rearrange("p t e h -> p t (e h)"),
                        wf[:, :, 0], wf[:, :, 1],
                    )
                    # pair products (broadcast APs; 3 free dims max per op)
                    prod = work.tile([128, 2, T, 8, H], BF16, tag="prod")
                    for i in range(2):
                        nc.vector.tensor_mul(
                            prod[:, i],
                            X[:, i, 0:T, :].unsqueeze(2).broadcast_to([128, T, 8, H]),
                            X[:, i, T:NE, :].unsqueeze(1).broadcast_to([128, T, 8, H]),
                        )
                    # head-sum 12 -> 6 -> 2 (last 2-fold rides the c matmul M)
                    h6 = work.tile([128, 2, NP, 6], BF16, tag="h6")
                    pv = prod.rearrange("p t s o h -> p t (s o) h")
                    nc.vector.tensor_add(h6, pv[:, :, :, 0:6], pv[:, :, :, 6:12])
                    Ad = A_sb[:, 2 * d:2 * d + 2]
                    nc.vector.tensor_add(Ad, h6[:, :, :, 0:2], h6[:, :, :, 2:4])
                    nc.vector.tensor_add(Ad, Ad, h6[:, :, :, 4:6])

                    # context matmul accumulation (M = 128 = (pair, h2))
                    for i in range(2):
                        t = 2 * d + i
                        ps0, ps1 = (c_psA0, c_psA1) if t < 8 else (c_psB0, c_psB1)
                        lhs = A_sb[:, t].rearrange("p n h -> p (n h)")
                        sq = seq_sbs[t // 4][:, t % 4]
                        nc.tensor.matmul(out=ps0, lhsT=lhs, rhs=sq[:, 0:384],
                                         start=(t % 8 == 0), stop=(t % 8 == 7))
                        nc.tensor.matmul(out=ps1, lhsT=lhs, rhs=sq[:, 384:769],
                                         start=(t % 8 == 0), stop=(t % 8 == 7))

                    if d == 1:   # entity-score DVE folds, mid-stream
                        sc_e_sb = consts.tile([RN, 2 * NP], F32)
                        nc.scalar.copy(sc_e_sb, sc_e)
                        eSO2 = consts.tile([RN, 2, NE, 2], F32)
                        eSO = consts.tile([RN, 2, NE], F32)
                        v = sc_e_sb.rearrange("r (x e w) -> r x e w", x=2, w=W)
                        nc.vector.tensor_add(eSO2, v[:, :, :, 0:2], v[:, :, :, 2:4])
                        nc.vector.tensor_add(eSO, eSO2[:, :, :, 0], eSO2[:, :, :, 1])
                        eSum = consts.tile([RN, T, 8], F32)   # 0.25*(eS[s]+eO[o])
                        nc.vector.scalar_tensor_tensor(
                            out=eSum,
                            in0=eSO[:, 0, 0:T].unsqueeze(2).broadcast_to([RN, T, 8]),
                            scalar=1.0,
                            in1=eSO[:, 1, T:NE].unsqueeze(1).broadcast_to([RN, T, 8]),
                            op0=mybir.AluOpType.mult, op1=mybir.AluOpType.add,
                        )
                        nc.vector.tensor_scalar_mul(eSum, eSum, 0.25)

                    if d == 3:   # A half: evacuate, fold, and score mid-chain
                        nc.scalar.copy(c2[:, 0, 0:384], c_psA0)
                        nc.scalar.copy(c2[:, 0, 384:769], c_psA1)
                        cT_psA = psT.tile([128, 6, NP], F32, tag="cT")
                        for k in range(6):
                            nc.tensor.matmul(out=cT_psA[:, k, :],
                                             lhsT=c2[:, 0, 128 * k:128 * (k + 1)],
                                             rhs=foldI)
                        nc.scalar.copy(cT_fA, cT_psA)
                        for k in range(6):
                            nc.tensor.matmul(out=sc_c, lhsT=relq_sb[:, 12 + k, :],
                                             rhs=cT_fA[:, k, :],
                                             start=(k == 0), stop=False)

                # --- tail: B half, 1/q, final assembly
                nc.scalar.copy(c2[:, 1, 0:384], c_psB0)
                nc.vector.tensor_copy(c2[:, 1, 384:769], c_psB1)
                # q chain (parallel to the big fold): q = qA + qB, transposed
                # and pair-folded by one tiny matmul against foldI
                qsum = consts.tile([128, 1], BF16)
                nc.vector.tensor_add(qsum, c2[:, 0, 768:769], c2[:, 1, 768:769])
                qT_ps = psT.tile([1, NP], F32, tag="qT")
                nc.tensor.matmul(out=qT_ps, lhsT=qsum, rhs=foldI)
                rq_row = consts.tile([1, NP], F32)
                nc.vector.reciprocal_approx_fast(rq_row, qT_ps)
                # B-half fold + score accumulation
                cT_psB = psT.tile([128, 6, NP], F32, tag="cT")
                for k in range(6):
                    nc.tensor.matmul(out=cT_psB[:, k, :],
                                     lhsT=c2[:, 1, 128 * k:128 * (k + 1)],
                                     rhs=foldI)
                nc.vector.tensor_copy(cT_fB, cT_psB)
                for k in range(6):
                    nc.tensor.matmul(out=sc_c, lhsT=relq_sb[:, 12 + k, :],
                                     rhs=cT_fB[:, k, :],
                                     start=False, stop=(k == 5))
                rq_ps = psT.tile([RN, NP], F32, tag="qT")
                nc.tensor.matmul(out=rq_ps, lhsT=ones_r, rhs=rq_row)
                rq_rep = consts.tile([RN, NP], BF16)
                nc.scalar.copy(rq_rep, rq_ps)
                fin_T = consts.tile([RN, NP], F32)
                # (c_scores * rq) + precomputed 0.25*(eS[s(p)] + eO[o(p)])
                nc.vector.tensor_mul(fin_T, sc_c, rq_rep)
                fin_T2 = consts.tile([RN, NP], F32)
                nc.vector.tensor_add(fin_T2.rearrange("r (s o) -> r s o", s=T),
                                     fin_T.rearrange("r (s o) -> r s o", s=T),
                                     eSum)

                # transpose to [p, rn], NOTA max, assemble [p, 1+R]
                finT_ps = psT.tile([NP, RN], F32, tag="fT")
                nc.tensor.transpose(finT_ps, fin_T2, id_f32)
                res = consts.tile([NP, R + 1], F32)
                nc.vector.reduce_max(res[:, 0:1], finT_ps[:, R:RN],
                                     axis=mybir.AxisListType.X)
                nc.scalar.copy(res[:, 1:R + 1], finT_ps[:, 0:R])
                nc.sync.dma_start(out=out[:, :], in_=res)

    return nc


def _host_shards(sequence_output, attention, relation_embeddings, nota_embeddings,
                 span_starts):
    rel_all = np.concatenate(
        [np.asarray(relation_embeddings, np.float32),
         np.asarray(nota_embeddings, np.float32)], axis=0)          # [77, 2304]
    relq = np.ascontiguousarray(
        rel_all.T.reshape(18, 128, RN).transpose(1, 0, 2).reshape(128, 18 * RN)
    ).astype(NP_BF16)

    in_maps = []
    for c in range(NCORES):
        b, g = divmod(c, 4)
        obj = list(range(8)) if g == 0 else list(range(8 * g, 8 * g + 8))
        ents = list(range(T)) + obj
        rows = np.concatenate(
            [np.arange(span_starts[b, e], span_starts[b, e] + W) for e in ents]
        )
        # attention rows -> [l, w, e, h] -> [128, (t w e h)]
        att_rows = attention[b][:, rows, :]                          # [H, 64, L]
        att_t = att_rows.reshape(H, NE, W, L).transpose(3, 2, 1, 0)  # [L, w, e, h]
        attq = (att_t.reshape(NT, 128, W * NE * H).transpose(1, 0, 2)
                .reshape(128, NT * W * NE * H)).astype(NP_BF16)
        # seq tiles + ones column -> [128, (t, 769)]
        st = sequence_output[b].reshape(NT, 128, D)
        st = np.concatenate([st, np.ones((NT, 128, 1), np.float32)], axis=2)
        seqq = st.transpose(1, 0, 2).reshape(128, NT * (D + 1)).astype(NP_BF16)
        # span rows transposed -> [d, row] chunks [128, (6, 64)]
        spT = sequence_output[b][rows].T.reshape(6, 128, NP).transpose(1, 0, 2)
        spansT = spT.reshape(128, 6 * NP).astype(NP_BF16)
        in_maps.append({
            "attq": np.ascontiguousarray(attq),
            "seqq": np.ascontiguousarray(seqq),
            "spansT": np.ascontiguousarray(spansT),
            "relq": relq,
        })
    return in_maps


def kernel(sequence_output, attention, relation_embeddings, nota_embeddings,
           span_starts):
    global LAST_RESULTS
    sequence_output = np.asarray(sequence_output, np.float32)
    attention = np.asarray(attention, np.float32)
    span_starts = np.asarray(span_starts)

    in_maps = _host_shards(sequence_output, attention, relation_embeddings,
                           nota_embeddings, span_starts)

    nc = _build_program()
    nc.finalize()
    LAST_RESULTS = run_bass_kernel_spmd(nc, in_maps, core_ids=list(range(NCORES)))

    out = np.zeros((B, len(ALL_PAIRS), R + 1), np.float32)
    for c in range(NCORES):
        b, g = divmod(c, 4)
        res = LAST_RESULTS.results[c]["out"]          # [64, 78], p = s*8+o_local
        idxs = GROUP_IDX[g]
        rows = [s * 8 + (o % 8) for (s, o) in (ALL_PAIRS[i] for i in idxs)]
        out[b, idxs, :] = res[rows]
    return out
